# revision 30
# baseline (speedup 1.0000x reference)
"""Trainium2 Bass kernel for the CTRF dense_cnn problem.

y[b,t,o] = b[o] + sum_{lag in [-10,40]} sum_d W[o,(lag+10)*64+d] * x[b,t-lag,d]

Strategy (8 NeuronCores, data-parallel over batch, 2 batches/core):
  - Pair adjacent timesteps: z[u] = concat(x[2u], x[2u+1]) in R^128, so each
    K=128 matmul covers two lags at once (full PE array instead of K=64).
  - The 51 lags become 52 stationary [128,128] matrices M_j = [Wblk_j; Wblk_{j-1}]
    (built on host); even j feed even output timesteps, odd j odd timesteps,
    each as a 26-tap conv over u with shifts s in [-20, 5].
  - x is PE-transposed on device to get features on partitions; outputs are
    PE-transposed back and stored with a stride-2 row interleave.
"""

import os
import sys

os.environ.setdefault("MYCRO_LOCAL_CACHE", "1")

for _p in ("/opt/trn_rl_repo", "/root/.axon_site/_ro/trn_rl_repo"):
    if os.path.isdir(_p) and _p not in sys.path:
        sys.path.insert(0, _p)

import numpy as np

import concourse.bass as bass  # noqa: E402
import concourse.mybir as mybir  # noqa: E402
import concourse.tile as tile  # noqa: E402
from concourse import bacc  # noqa: E402
from concourse.bass import ts  # noqa: E402
from concourse.bass_utils import run_bass_kernel_spmd  # noqa: E402
from concourse.masks import make_identity  # noqa: E402

N_CORES = 8
B, T, D, O = 16, 2048, 64, 128
NLAGS = 51
U = T // 2          # pair rows per batch
BPC = B // N_CORES  # batches per core
NJ = NLAGS + 1      # stationary matrices
PAD_L = 20          # zero cols left of z (covers shift -20)
PAD_R = 12          # zero cols right of z (need >= 5)
ZCOLS = PAD_L + U + PAD_R
CHUNK = 512         # psum free dim (one bank of fp32)
USE_F32R = True     # fp32r: 4x PE throughput, relaxed mantissa


def _shift_for(j):
    return (10 - j) // 2 if j % 2 == 0 else (11 - j) // 2


# 8 weight chunks: 4 covering even j (par 0, idx 0..25), 4 covering odd j.
WP_CHUNK_SIZES = [7, 7, 6, 6, 7, 7, 6, 6]


def _chunk_for(par, idx):
    base = 0 if par == 0 else 4
    cum = 0
    for c in range(base, base + 4):
        if idx < cum + WP_CHUNK_SIZES[c]:
            return c, idx - cum
        cum += WP_CHUNK_SIZES[c]
    raise ValueError(idx)


def _build():
    nc = bacc.Bacc(
        "TRN2", target_bir_lowering=False, debug=False, num_devices=N_CORES
    )
    f32 = mybir.dt.float32
    f32r = mybir.dt.float32r
    mm_dt = f32r if USE_F32R else f32

    x_d = nc.declare_dram_parameter("x", [BPC, T, D], f32, isOutput=False)
    # Stationary matrices [k, j, o] (partition k contiguous in DRAM), split
    # into 8 chunks (4 even-j, 4 odd-j) as separate params/tiles so the first
    # matmul only gates on ~430KB of weights.
    wp_ds = [
        nc.declare_dram_parameter(f"wp{c}", [128, n, O], mm_dt, isOutput=False)
        for c, n in enumerate(WP_CHUNK_SIZES)
    ]
    b_d = nc.declare_dram_parameter("bvec", [O, 1], f32, isOutput=False)
    y_d = nc.declare_dram_parameter("y", [BPC, T, O], f32, isOutput=True)

    with tile.TileContext(nc) as tc:
        with (
            tc.tile_pool(name="consts", bufs=1) as consts,
            tc.tile_pool(name="zt", bufs=2) as zt_pool,
            tc.tile_pool(name="xbig", bufs=2) as xbig_pool,
            tc.tile_pool(name="osb", bufs=2) as osb_pool,
            tc.tile_pool(name="ybig", bufs=2) as ybig_pool,
            tc.tile_pool(name="pacc", bufs=2, space="PSUM") as pacc_pool,
            tc.tile_pool(name="ptr", bufs=3, space="PSUM") as ptr_pool,
        ):
            ident = consts.tile([128, 128], f32)
            make_identity(nc, ident[:])

            # DMA order = consumption order: x(b0), even-j weights + bias,
            # x(b1), odd-j weights. Each wp chunk is its own tile so matmuls
            # gate on exactly the chunk they read.
            xbigs = []
            xb0 = xbig_pool.tile([128, U], f32, tag="xbig")
            xbigs.append(xb0)
            nc.sync.dma_start(
                xb0[:], x_d[0].rearrange("(p f) d -> p (f d)", p=128)
            )
            bias_sb = consts.tile([O, 1], f32)
            wp_sbs = []
            for c in range(8):
                if c == 1:
                    nc.sync.dma_start(bias_sb[:], b_d[:])
                if c == 4:
                    xb1 = xbig_pool.tile([128, U], f32, tag="xbig")
                    xbigs.append(xb1)
                    nc.sync.dma_start(
                        xb1[:], x_d[1].rearrange("(p f) d -> p (f d)", p=128)
                    )
                wp_sb = consts.tile(
                    [128, WP_CHUNK_SIZES[c], O], mm_dt, tag=f"wp{c}"
                )
                nc.sync.dma_start(wp_sb[:], wp_ds[c][:])
                wp_sbs.append(wp_sb)

            def lhs_for(par, idx):
                c, off = _chunk_for(par, idx)
                return wp_sbs[c][:, off, :]

            def emit_transposes(bb, zt):
                # x[b] viewed [128, 1024] is per-partition contiguous; slice
                # i's PE transpose yields zT columns u = 8p + i, scattered by
                # a stride-8 DVE copy into zt.
                nc.vector.memset(zt[:, 0:PAD_L].bitcast(f32), 0.0)
                nc.vector.memset(zt[:, PAD_L + U :].bitcast(f32), 0.0)
                ztv = zt[:, PAD_L : PAD_L + U].rearrange("c (uo ui) -> c uo ui", ui=8)
                for i in range(8):
                    ptr_t = ptr_pool.tile([128, 128], f32)
                    nc.tensor.transpose(ptr_t[:], xbigs[bb][:, ts(i, 128)], ident[:])
                    nc.vector.tensor_copy(ztv[:, :, i], ptr_t[:])

            def emit_group(bb, zt, par, u0):
                yv = y_d[bb].rearrange("(i p two) o -> p i two o", two=2, p=128)
                pacc = pacc_pool.tile([128, CHUNK], f32)
                js = list(range(par, NJ, 2))
                for idx, j in enumerate(js):
                    c0 = PAD_L + u0 + _shift_for(j)
                    nc.tensor.matmul(
                        pacc[:],
                        lhs_for(par, idx),
                        zt[:, c0 : c0 + CHUNK],
                        start=(idx == 0),
                        stop=(idx == len(js) - 1),
                    )
                osb = osb_pool.tile([128, CHUNK], f32)
                nc.scalar.activation(
                    osb[:],
                    pacc[:],
                    mybir.ActivationFunctionType.Identity,
                    bias=bias_sb[:],
                )
                ybc = ybig_pool.tile([128, CHUNK // 128, O], f32)
                for i in range(CHUNK // 128):
                    ptr_t = ptr_pool.tile([128, 128], f32)
                    nc.tensor.transpose(ptr_t[:], osb[:, ts(i, 128)], ident[:])
                    nc.vector.tensor_copy(ybc[:, i, :], ptr_t[:])
                nc.sync.dma_start(yv[:, ts(u0 // CHUNK, 4), par, :], ybc[:])

            zt0 = zt_pool.tile([128, ZCOLS], mm_dt, tag="zt")
            zt1 = zt_pool.tile([128, ZCOLS], mm_dt, tag="zt")
            emit_transposes(0, zt0)
            for u0 in range(0, U, CHUNK):
                emit_group(0, zt0, 0, u0)
            emit_transposes(1, zt1)
            for u0 in range(0, U, CHUNK):
                emit_group(0, zt0, 1, u0)
            for par in range(2):
                for u0 in range(0, U, CHUNK):
                    emit_group(1, zt1, par, u0)
    nc.compile()
    return nc


_NC_CACHE = {}


def _get_program():
    if "nc" not in _NC_CACHE:
        _NC_CACHE["nc"] = _build()
    return _NC_CACHE["nc"]


def _prep_inputs(x, W, b):
    x = np.ascontiguousarray(x, dtype=np.float32)
    W = np.ascontiguousarray(W, dtype=np.float32)
    b = np.ascontiguousarray(b, dtype=np.float32)
    Wt = W.reshape(O, NLAGS, D).transpose(1, 2, 0)  # [j, d, o]
    wp = np.zeros((NJ, 128, O), dtype=np.float32)
    wp[:NLAGS, :D, :] = Wt
    wp[1:, D:, :] = Wt
    bvec = np.ascontiguousarray(b.reshape(O, 1))
    maps = []
    chunk_js = []
    for par in range(2):
        js = list(range(par, NJ, 2))
        cum = 0
        for c in range(4):
            n = WP_CHUNK_SIZES[par * 4 + c]
            chunk_js.append(js[cum : cum + n])
            cum += n
    wp_chunks = {
        f"wp{c}": np.ascontiguousarray(wp[js].transpose(1, 0, 2))
        for c, js in enumerate(chunk_js)
    }
    for c in range(N_CORES):
        m = {"x": x[c * BPC : (c + 1) * BPC], "bvec": bvec}
        m.update(wp_chunks)
        maps.append(m)
    return maps


def kernel(x, W, b):
    in_maps = _prep_inputs(x, W, b)
    res = run_bass_kernel_spmd(
        _get_program(), in_maps, core_ids=list(range(N_CORES))
    )
    return np.concatenate(
        [res.results[c]["y"] for c in range(N_CORES)], axis=0
    )


def _ensure_ntff_hook():
    """The agent image's antenv lacks axon_hooks, so run_bass_kernel_spmd's
    trace path degrades to no-profile. Seed an equivalent module backed by
    the ctypes NTFF profiler from trn_agent_boot."""
    try:
        from antenv.axon_hooks import get_axon_ntff_profile_hook

        if get_axon_ntff_profile_hook() is not None:
            return True
    except ImportError:
        pass
    try:
        import types

        site_dir = "/root/.axon_site"
        if site_dir not in sys.path and os.path.isdir(site_dir):
            sys.path.insert(0, site_dir)
        from trn_agent_boot.trn_boot import _ntff_profile_via_ctypes

        hook = _ntff_profile_via_ctypes("/opt/axon/libaxon_pjrt.so")
        if hook is None:
            return False
        mod = types.ModuleType("antenv.axon_hooks")
        mod.get_axon_ntff_profile_hook = lambda: hook
        mod.set_axon_ntff_profile_hook = lambda h: None
        sys.modules["antenv.axon_hooks"] = mod
        import antenv

        antenv.axon_hooks = mod
        return True
    except Exception:
        return False


def kernel_traced(x, W, b, **kwargs):
    """Like kernel() but requests an NTFF trace; returns (y, BassKernelResults).

    Dev-loop only (test.py); the graded kernel() path never traces. The
    artifact upload is stubbed out since this container has no bucket access.
    """
    _ensure_ntff_hook()
    from concourse import bass_utils as _bu

    in_maps = _prep_inputs(x, W, b)
    orig_upload = _bu.upload_artifacts
    _bu.upload_artifacts = lambda tmpdir: f"local:{tmpdir}"
    try:
        res = run_bass_kernel_spmd(
            _get_program(), in_maps, core_ids=list(range(N_CORES)), trace=True, **kwargs
        )
    finally:
        _bu.upload_artifacts = orig_upload
    y = np.concatenate([res.results[c]["y"] for c in range(N_CORES)], axis=0)
    return y, res
